# revision 3
# baseline (speedup 1.0000x reference)
"""Trainium2 Bass kernel for a dense transformer decoder block.

Strategy (8 NeuronCores, tensor-parallel a la Megatron), v2 — fully
software-pipelined at 512-token chunk granularity:
  - heads sharded across cores (H/8 heads each) for attention,
    FFN hidden dim sharded (HID/8 each).
  - x shipped host-side as bf16 in transposed [B, D, S] layout; all
    matmuls contract over the partition dim in bf16.
  - The mid-block AllReduce (h = x + attn) is chunked per 512 tokens
    (bf16) and each chunk's FFN is interleaved back into the attention
    token loop two chunks later, so attention matmuls, AllReduce
    traffic, and FFN matmuls all overlap; rmsnorm stats for chunk c+1
    are computed during chunk c to keep reciprocal latency off the
    critical path.
  - Residual x is folded into the AllReduce as x/8; the final residual
    h is folded into the per-chunk ReduceScatter as h/8; the RS output
    IS the final result, sharded over D rows across cores.  Host
    reassembles + transposes.
  - Causality is not hardcoded: the mask input is classified host-side
    into skip / plain / mixed 128x512 blocks; mixed tiles are shipped
    as constants (4 distinct tiles for a causal mask).
"""

import os
import sys

try:  # the axon sitecustomize usually provides concourse already
    import concourse.bass  # noqa: F401
except ImportError:  # pragma: no cover
    sys.path.insert(0, "/opt/trn_rl_repo")

from contextlib import ExitStack

import ml_dtypes
import numpy as np

import concourse.bacc as bacc
import concourse.tile as tile
from concourse import mybir
from concourse.bass_utils import run_bass_kernel_spmd
from concourse.masks import make_identity

F32 = mybir.dt.float32
BF16 = mybir.dt.bfloat16
N_CORES = 8
P = 128
QW = 512  # q-tile / token-chunk width
EPS = 1e-6
AF = mybir.ActivationFunctionType
BF16_NP = ml_dtypes.bfloat16


def ts(i, w):
    return slice(i * w, (i + 1) * w)


def _classify_mask(mask, S):
    """mask: [S, S] additive (q, k). Returns (table, tiles).
    table[(kt, j)] = 'skip' | 'plain' | int mask-tile index.
    tiles: list of [128, QW] float32 arrays in scoresT ([k, q]) layout."""
    table = {}
    tiles = []
    keys = {}
    for j in range(S // QW):
        for kt in range(S // P):
            sub = mask[ts(j, QW), ts(kt, P)]  # [q, k]
            if np.all(sub <= -1e8):
                table[(kt, j)] = "skip"
            elif np.all(sub == 0.0):
                table[(kt, j)] = "plain"
            else:
                t = np.ascontiguousarray(sub.T.astype(np.float32))  # [k, q]
                key = t.tobytes()
                if key not in keys:
                    keys[key] = len(tiles)
                    tiles.append(t)
                table[(kt, j)] = keys[key]
    return table, tiles


def build_program(B, S, D, H, HID, mask_table, n_mask):
    HD = 128
    assert D == (D // P) * P and H * HD == D
    HPC = H // N_CORES            # heads per core
    assert HPC * N_CORES == H
    C = D // P                    # contraction chunks over D
    S_TILES = S // QW             # q tiles per batch
    KT = S // P                   # k tiles per batch
    T = B * S                     # total tokens
    OC = HPC                      # wo input-channel chunks
    HIDC = HID // N_CORES // P    # hidden tiles per core
    NCH = T // QW                 # token chunks (pipeline granularity)
    DS = D // N_CORES             # output row shard per core
    W2Q = 4                       # w2 streamed in quarters along D
    LAG = 2                       # ffn(c) runs at iteration c + LAG

    nc = bacc.Bacc(trn_type="TRN2", num_devices=N_CORES)

    xt = nc.dram_tensor("xt", [B, D, S], BF16, kind="ExternalInput").ap()
    wq = nc.dram_tensor("wq", [C, P, HPC * HD], BF16, kind="ExternalInput").ap()
    wk = nc.dram_tensor("wk", [C, P, HPC * HD], BF16, kind="ExternalInput").ap()
    wv = nc.dram_tensor("wv", [C, P, HPC * HD], BF16, kind="ExternalInput").ap()
    wo = nc.dram_tensor("wo", [OC, P, D], BF16, kind="ExternalInput").ap()
    w1 = nc.dram_tensor("w1", [C, P, HIDC * P], BF16, kind="ExternalInput").ap()
    w2 = nc.dram_tensor("w2", [HIDC, P, D], BF16, kind="ExternalInput").ap()
    mk = None
    if n_mask:
        mk = nc.dram_tensor("mk", [n_mask, P, QW], BF16, kind="ExternalInput").ap()

    ar_in = [nc.dram_tensor(f"ar_in{k}", [D, QW], BF16) for k in range(NCH)]
    ar_out = [
        nc.dram_tensor(f"ar_out{k}", [D, QW], BF16, addr_space="Shared")
        for k in range(NCH)
    ]
    rs_in = [nc.dram_tensor(f"rs_in{k}", [D, QW], F32) for k in range(NCH)]
    rs_out = [nc.dram_tensor(f"rs_out{k}", [DS, QW], F32) for k in range(NCH)]
    outs = [
        nc.dram_tensor(f"out{k}", [DS, QW], F32, kind="ExternalOutput")
        for k in range(NCH)
    ]

    groups = [list(range(N_CORES))]

    with tile.TileContext(nc) as tc, ExitStack() as ctx:
        const = ctx.enter_context(tc.tile_pool(name="const", bufs=1))
        wpool = ctx.enter_context(tc.tile_pool(name="w", bufs=1))
        w1p = ctx.enter_context(tc.tile_pool(name="w1p", bufs=3))
        w2p = ctx.enter_context(tc.tile_pool(name="w2p", bufs=2))
        kvp = ctx.enter_context(tc.tile_pool(name="kv", bufs=1))
        xbp = ctx.enter_context(tc.tile_pool(name="xb", bufs=2))
        htp = ctx.enter_context(tc.tile_pool(name="ht", bufs=2))
        hnp = ctx.enter_context(tc.tile_pool(name="hn", bufs=1))
        upp = ctx.enter_context(tc.tile_pool(name="up", bufs=1))
        qap = ctx.enter_context(tc.tile_pool(name="qa", bufs=2))
        expp = ctx.enter_context(tc.tile_pool(name="exp", bufs=3))
        sqp = ctx.enter_context(tc.tile_pool(name="sq", bufs=2))
        evp = ctx.enter_context(tc.tile_pool(name="ev", bufs=2))
        stp = ctx.enter_context(tc.tile_pool(name="st", bufs=2))
        psum = ctx.enter_context(tc.tile_pool(name="psum", bufs=1, space="PSUM"))

        ones_f32 = const.tile([P, P], F32)
        nc.vector.memset(ones_f32[:], 1.0)
        ones = const.tile([P, P], BF16)
        nc.vector.tensor_copy(ones[:], ones_f32[:])
        eps_p1 = const.tile([P, 1], F32)
        nc.vector.memset(eps_p1[:], EPS)
        ident = const.tile([P, P], F32)
        make_identity(nc, ident[:])

        # resident attention weights + mask tiles
        mtiles = None
        if n_mask:
            mtiles = wpool.tile([P, n_mask, QW], BF16, tag="mk")
            nc.sync.dma_start(mtiles[:], mk.rearrange("n p q -> p n q"))
        wq_sb = wpool.tile([P, C, HPC * HD], BF16, tag="wq")
        nc.sync.dma_start(wq_sb[:], wq.rearrange("c p o -> p c o"))
        wk_sb = wpool.tile([P, C, HPC * HD], BF16, tag="wk")
        nc.sync.dma_start(wk_sb[:], wk.rearrange("c p o -> p c o"))
        wv_sb = wpool.tile([P, C, HPC * HD], BF16, tag="wv")
        nc.sync.dma_start(wv_sb[:], wv.rearrange("c p o -> p c o"))
        wo_sb = wpool.tile([P, OC, D], BF16, tag="wo")
        nc.sync.dma_start(wo_sb[:], wo.rearrange("c p o -> p c o"))

        # pipeline state carried across iterations
        state = {}

        def load_xb(c):
            b, j = divmod(c, S_TILES)
            xb = xbp.tile([P, C, QW], BF16, tag="xb")
            nc.sync.dma_start(
                xb[:],
                xt[b].rearrange("(c p) t -> p c t", p=P)[:, :, ts(j, QW)],
            )
            state[("xb", c)] = xb

        def stats(src, want_rcol):
            """rmsnorm stats from a [P, C, QW] bf16 tile; returns rinv
            ([P, QW] f32, partition-replicated) and optionally rcol
            ([P, QW // P], token-partition layout)."""
            cs = psum.tile([P, QW], F32, tag="cs", bufs=1)
            for cc in range(C):
                sq = sqp.tile([P, QW], BF16, tag="sq")
                nc.scalar.activation(sq[:], src[:, cc, :], AF.Square)
                nc.tensor.matmul(
                    cs[:], ones[:], sq[:], start=(cc == 0), stop=(cc == C - 1)
                )
            rms = stp.tile([P, QW], F32, tag="rms")
            nc.scalar.activation(
                rms[:], cs[:], AF.Sqrt, bias=eps_p1[:], scale=1.0 / D
            )
            rinv = stp.tile([P, QW], F32, tag="rinv")
            nc.vector.reciprocal(rinv[:], rms[:])
            rcol = None
            if want_rcol:
                rcol = stp.tile([P, QW // P], F32, tag="rcol")
                for sub in range(QW // P):
                    tp = psum.tile([P, QW], F32, tag="score", bufs=2)
                    nc.tensor.transpose(tp[:, :P], rinv[:, ts(sub, P)], ident[:])
                    nc.vector.tensor_copy(rcol[:, sub : sub + 1], tp[:, 0:1])
            return rinv, rcol

        def att_stats(c):
            rinv, rcol = stats(state[("xb", c)], True)
            state[("rinv", c)] = rinv
            state[("rcol", c)] = rcol

        def attention(c):
            b, j = divmod(c, S_TILES)
            if j == 0:
                state["kT"] = kvp.tile([P, HPC, S], BF16, tag="kT", name="kT")
                state["vN"] = kvp.tile(
                    [P, KT, HPC * HD], BF16, tag="vN", name="vN"
                )
            kT, vN = state["kT"], state["vN"]
            xb = state.pop(("xb", c))
            rinv = state.pop(("rinv", c))
            rcol = state.pop(("rcol", c))

            # q/k projections; rinv folded in at eviction
            qTs = qap.tile([P, HPC, QW], BF16, tag="qT", bufs=1)
            for h in range(HPC):
                for w_sb, is_q in ((wq_sb, True), (wk_sb, False)):
                    pp = psum.tile([P, QW], F32, tag="mm", bufs=2)
                    for cc in range(C):
                        nc.tensor.matmul(
                            pp[:],
                            w_sb[:, cc, ts(h, HD)],
                            xb[:, cc, :],
                            start=(cc == 0),
                            stop=(cc == C - 1),
                        )
                    if is_q:
                        nc.vector.tensor_mul(qTs[:, h, :], pp[:], rinv[:])
                    else:
                        nc.vector.tensor_mul(kT[:, h, ts(j, QW)], pp[:], rinv[:])
            # v in natural layout; rinv via per-partition scalar
            for sub in range(QW // P):
                pv = psum.tile([P, QW], F32, tag="mm", bufs=2)
                for cc in range(C):
                    nc.tensor.matmul(
                        pv[:, : HPC * HD],
                        xb[:, cc, ts(sub, P)],
                        wv_sb[:, cc, :],
                        start=(cc == 0),
                        stop=(cc == C - 1),
                    )
                nc.vector.tensor_scalar_mul(
                    vN[:, j * (QW // P) + sub, :],
                    pv[:, : HPC * HD],
                    rcol[:, sub : sub + 1],
                )

            # flash-style attention for this q-tile
            attnT = qap.tile([P, HPC, QW], BF16, tag="attnT", bufs=2)
            for h in range(HPC):
                kts = [kt for kt in range(KT) if mask_table[(kt, j)] != "skip"]
                pa = psum.tile([P, QW], F32, tag="pv", bufs=2)
                den = psum.tile([P, QW], F32, tag="den", bufs=1)
                n_k = len(kts)
                exs = [None] * n_k

                def _den_pv(i):
                    kt = kts[i]
                    nc.tensor.matmul(
                        den[:], ones[:], exs[i][:],
                        start=(i == 0), stop=(i == n_k - 1),
                    )
                    nc.tensor.matmul(
                        pa[:],
                        vN[:, kt, ts(h, HD)],
                        exs[i][:],
                        start=(i == 0),
                        stop=(i == n_k - 1),
                    )

                for i, kt in enumerate(kts):
                    msc = psum.tile([P, QW], F32, tag="score", bufs=2)
                    nc.tensor.matmul(
                        msc[:],
                        kT[:, h, ts(kt, P)],
                        qTs[:, h, :],
                        start=True,
                        stop=True,
                    )
                    ex = expp.tile([P, QW], BF16, tag="exp", bufs=3)
                    mt = mask_table[(kt, j)]
                    if mt == "plain":
                        nc.scalar.activation(ex[:], msc[:], AF.Exp)
                    else:
                        nc.vector.tensor_add(ex[:], msc[:], mtiles[:, mt, :])
                        nc.scalar.activation(ex[:], ex[:], AF.Exp)
                    exs[i] = ex
                    if i > 0:
                        _den_pv(i - 1)
                _den_pv(n_k - 1)
                rec = stp.tile([P, QW], F32, tag="rec")
                nc.vector.reciprocal(rec[:], den[:])
                nc.vector.tensor_mul(attnT[:, h, :], pa[:], rec[:])

            # wo partial + x/8 -> AR input
            for ot in range(C):
                po = psum.tile([P, QW], F32, tag="mm", bufs=2)
                for oc in range(OC):
                    nc.tensor.matmul(
                        po[:],
                        wo_sb[:, oc, ts(ot, P)],
                        attnT[:, oc, :],
                        start=(oc == 0),
                        stop=(oc == OC - 1),
                    )
                ev = evp.tile([P, QW], BF16, tag="eva")
                nc.vector.scalar_tensor_tensor(
                    ev[:], xb[:, ot, :], 1.0 / N_CORES, po[:],
                    op0=mybir.AluOpType.mult, op1=mybir.AluOpType.add,
                )
                nc.gpsimd.dma_start(ar_in[c].ap()[ts(ot, P), :], ev[:])

        def load_ht(c):
            ht = htp.tile([P, C, QW], BF16, tag="ht")
            nc.sync.dma_start(
                ht[:], ar_out[c].ap().rearrange("(c p) t -> p c t", p=P)
            )
            state[("ht", c)] = ht

        def ffn_stats(c):
            ht = state[("ht", c)]
            r2, _ = stats(ht, False)
            hn = hnp.tile([P, C, QW], BF16, tag="hn")
            for cc in range(C):
                nc.vector.tensor_mul(hn[:, cc, :], ht[:, cc, :], r2[:])
            state[("hn", c)] = hn

        def load_w1(i):
            w1t = w1p.tile([P, C, P], BF16, tag="w1s")
            nc.sync.dma_start(
                w1t[:], w1.rearrange("c p o -> p c o")[:, :, ts(i, P)]
            )
            return w1t

        def load_w2(q):
            w2t = w2p.tile([P, HIDC, QW], BF16, tag="w2q")
            nc.sync.dma_start(
                w2t[:], w2.rearrange("c p o -> p c o")[:, :, ts(q, QW)]
            )
            return w2t

        def ffn_body(c):
            ht = state.pop(("ht", c))
            hn = state.pop(("hn", c))
            w1ts = [load_w1(0), load_w1(1)]
            w2ts = [load_w2(0)]
            up = upp.tile([P, HIDC, QW], BF16, tag="up")
            for ht_i in range(HIDC):
                if ht_i + 2 < HIDC:
                    w1ts.append(load_w1(ht_i + 2))
                pu = psum.tile([P, QW], F32, tag="mm", bufs=2)
                for cc in range(C):
                    nc.tensor.matmul(
                        pu[:],
                        w1ts[ht_i][:, cc, :],
                        hn[:, cc, :],
                        start=(cc == 0),
                        stop=(cc == C - 1),
                    )
                nc.scalar.activation(up[:, ht_i, :], pu[:], AF.Relu)
            for q in range(W2Q):
                if q + 1 < W2Q:
                    w2ts.append(load_w2(q + 1))
                for o4 in range(C // W2Q):
                    ot = q * (C // W2Q) + o4
                    pd = psum.tile([P, QW], F32, tag="mm", bufs=2)
                    for hc in range(HIDC):
                        nc.tensor.matmul(
                            pd[:],
                            w2ts[q][:, hc, ts(o4, P)],
                            up[:, hc, :],
                            start=(hc == 0),
                            stop=(hc == HIDC - 1),
                        )
                    ev = evp.tile([P, QW], F32, tag="evf")
                    nc.vector.scalar_tensor_tensor(
                        ev[:], ht[:, ot, :], 1.0 / N_CORES, pd[:],
                        op0=mybir.AluOpType.mult, op1=mybir.AluOpType.add,
                    )
                    nc.gpsimd.dma_start(rs_in[c].ap()[ts(ot, P), :], ev[:])

        # ---- software-pipelined main loop ----
        load_xb(0)
        att_stats(0)
        for c in range(NCH + LAG):
            if c + 1 < NCH:
                load_xb(c + 1)
            if 0 <= c - 1 < NCH:
                load_ht(c - 1)
            if c < NCH:
                attention(c)
                nc.gpsimd.collective_compute(
                    "AllReduce",
                    mybir.AluOpType.add,
                    replica_groups=groups,
                    ins=[ar_in[c].ap().opt()],
                    outs=[ar_out[c].ap().opt()],
                )
            if c + 1 < NCH:
                att_stats(c + 1)
            if c - LAG >= 0:
                f = c - LAG
                ffn_body(f)
                nc.gpsimd.collective_compute(
                    "ReduceScatter",
                    mybir.AluOpType.add,
                    replica_groups=groups,
                    ins=[rs_in[f].ap().opt()],
                    outs=[rs_out[f].ap().opt()],
                )
                nc.sync.dma_start(outs[f].ap(), rs_out[f].ap())
            if 0 <= c - 1 < NCH:
                ffn_stats(c - 1)

    nc.compile()
    return nc, NCH, DS


_CACHE = {}
LAST_RESULT = None


def _get_program(B, S, D, H, HID, mask_table, n_mask, mask_key):
    key = (B, S, D, H, HID, mask_key)
    if key not in _CACHE:
        _CACHE[key] = build_program(B, S, D, H, HID, mask_table, n_mask)
    return _CACHE[key]


def kernel(x, mask, wq, wk, wv, wo, w1, w2, attn_norm_w, ffn_norm_w):
    x = np.asarray(x, dtype=np.float32)
    mask = np.asarray(mask, dtype=np.float32)
    wq, wk, wv, wo = (np.asarray(a, dtype=np.float32) for a in (wq, wk, wv, wo))
    w1, w2 = np.asarray(w1, dtype=np.float32), np.asarray(w2, dtype=np.float32)
    attn_norm_w = np.asarray(attn_norm_w, dtype=np.float32)
    ffn_norm_w = np.asarray(ffn_norm_w, dtype=np.float32)

    B, S, D = x.shape
    H = D // 128  # HD is fixed at 128 (= SBUF partition count)
    HID = w1.shape[0]
    HD = D // H
    HPC = H // N_CORES
    C = D // P
    HIDC = HID // N_CORES // P

    mask_table, mtiles_np = _classify_mask(
        np.broadcast_to(mask, (1, 1, S, S))[0, 0], S
    )
    mask_key = hash(tuple(sorted((k, str(v)) for k, v in mask_table.items())))
    nc, NCH, DS = _get_program(
        B, S, D, H, HID, mask_table, len(mtiles_np), mask_key
    )

    # ---- host-side prep ----
    xt = np.ascontiguousarray(x.transpose(0, 2, 1)).astype(BF16_NP)  # [B, D, S]
    wq_f = (wq * attn_norm_w[None, :]) / np.sqrt(HD)
    wk_f = wk * attn_norm_w[None, :]
    w1_f = w1 * ffn_norm_w[None, :]

    in_maps = []
    for c in range(N_CORES):
        hs = slice(c * HPC * HD, (c + 1) * HPC * HD)
        qs = np.ascontiguousarray(wq_f[hs].T).reshape(C, P, HPC * HD).astype(BF16_NP)
        ks = np.ascontiguousarray(wk_f[hs].T).reshape(C, P, HPC * HD).astype(BF16_NP)
        vs = np.ascontiguousarray(wv[hs].T).reshape(C, P, HPC * HD).astype(BF16_NP)
        os_ = np.ascontiguousarray(wo[:, hs].T).reshape(HPC, P, D).astype(BF16_NP)
        fs = slice(c * HIDC * P, (c + 1) * HIDC * P)
        w1s = np.ascontiguousarray(w1_f[fs].T).reshape(C, P, HIDC * P).astype(BF16_NP)
        w2r = (
            np.ascontiguousarray(w2[:, fs].T)
            .reshape(HIDC, P, D)
            .astype(BF16_NP)
        )
        m = {
            "xt": xt,
            "wq": qs,
            "wk": ks,
            "wv": vs,
            "wo": os_,
            "w1": w1s,
            "w2": w2r,
        }
        if len(mtiles_np):
            m["mk"] = np.stack(mtiles_np).astype(BF16_NP)
        in_maps.append(m)

    trace = os.environ.get("KTRACE", "0") == "1"
    res = run_bass_kernel_spmd(nc, in_maps, list(range(N_CORES)), trace=trace)
    global LAST_RESULT
    LAST_RESULT = res

    out_T = np.empty((D, B * S), dtype=np.float32)
    for r_ in range(N_CORES):
        for k in range(NCH):
            out_T[r_ * DS : (r_ + 1) * DS, k * QW : (k + 1) * QW] = res.results[
                r_
            ][f"out{k}"]
    return np.ascontiguousarray(out_T.reshape(D, B, S).transpose(1, 2, 0))


# revision 5
# speedup vs baseline: 1.0681x; 1.0681x over previous
"""Trainium2 Bass kernel for a dense transformer decoder block.

Strategy (8 NeuronCores, tensor-parallel a la Megatron), v3 — fully
software-pipelined at 512-token chunk granularity:
  - heads sharded across cores (H/8 heads each) for attention,
    FFN hidden dim sharded (HID/8 each).
  - x shipped host-side as bf16 in transposed [B, D, S] layout; all
    matmuls contract over the partition dim in bf16.
  - The mid-block AllReduce (h = x + attn) is chunked per 512 tokens
    (bf16) and each chunk's FFN is interleaved back into the attention
    token loop two chunks later, so attention matmuls, AllReduce
    traffic, and FFN matmuls all overlap.
  - rmsnorm stats for the NEXT chunk are fused into the wo / w2
    eviction loops (one square + ones-matmul paired with each output
    chain) so the scalar engine never rate-limits the PE; the
    sqrt/reciprocal tails are scheduled behind dense matmul stretches.
  - softmax denominators accumulate on the DVE (one ones-matmul per
    head instead of one per k-block).
  - masking is multiplicative: exp(mask) (exact 0/1 in bf16) applied
    after the exp, so the activation reads straight from PSUM.
  - Residual x is folded into the AllReduce as x/8; the final residual
    h is folded into the per-chunk bf16 ReduceScatter as h/8; the RS
    output IS the final result, sharded over D rows across cores.
    Host reassembles + transposes.
  - Causality is not hardcoded: the mask input is classified host-side
    into skip / plain / mixed 128x512 blocks.
"""

import os
import sys

try:  # the axon sitecustomize usually provides concourse already
    import concourse.bass  # noqa: F401
except ImportError:  # pragma: no cover
    sys.path.insert(0, "/opt/trn_rl_repo")

from contextlib import ExitStack

import ml_dtypes
import numpy as np

import concourse.bacc as bacc
import concourse.tile as tile
from concourse import mybir
from concourse.bass_utils import run_bass_kernel_spmd
from concourse.masks import make_identity

F32 = mybir.dt.float32
BF16 = mybir.dt.bfloat16
N_CORES = 8
P = 128
QW = 512  # q-tile / token-chunk width
EPS = 1e-6
AF = mybir.ActivationFunctionType
BF16_NP = ml_dtypes.bfloat16


def ts(i, w):
    return slice(i * w, (i + 1) * w)


def _classify_mask(mask, S):
    """mask: [S, S] additive (q, k). Returns (table, tiles).
    table[(kt, j)] = 'skip' | 'plain' | int mask-tile index.
    tiles: list of [128, QW] float32 arrays of exp(mask) in scoresT
    ([k, q]) layout (multiplicative masking)."""
    table = {}
    tiles = []
    keys = {}
    for j in range(S // QW):
        for kt in range(S // P):
            sub = mask[ts(j, QW), ts(kt, P)]  # [q, k]
            if np.all(sub <= -1e8):
                table[(kt, j)] = "skip"
            elif np.all(sub == 0.0):
                table[(kt, j)] = "plain"
            else:
                t = np.ascontiguousarray(np.exp(sub.T.astype(np.float64)))
                t = t.astype(np.float32)  # [k, q]
                key = t.tobytes()
                if key not in keys:
                    keys[key] = len(tiles)
                    tiles.append(t)
                table[(kt, j)] = keys[key]
    return table, tiles


def build_program(B, S, D, H, HID, mask_table, n_mask):
    HD = 128
    assert D == (D // P) * P and H * HD == D
    HPC = H // N_CORES            # heads per core
    assert HPC * N_CORES == H
    C = D // P                    # contraction chunks over D
    S_TILES = S // QW             # q tiles per batch
    KT = S // P                   # k tiles per batch
    T = B * S                     # total tokens
    OC = HPC                      # wo input-channel chunks
    HIDC = HID // N_CORES // P    # hidden tiles per core
    NCH = T // QW                 # token chunks (pipeline granularity)
    DS = D // N_CORES             # output row shard per core
    W2Q = 4                       # w2 streamed in quarters along D
    LAG = 2                       # ffn(c) runs at iteration c + LAG

    nc = bacc.Bacc(trn_type="TRN2", num_devices=N_CORES)

    xt = nc.dram_tensor("xt", [B, D, S], BF16, kind="ExternalInput").ap()
    wq = nc.dram_tensor("wq", [C, P, HPC * HD], BF16, kind="ExternalInput").ap()
    wk = nc.dram_tensor("wk", [C, P, HPC * HD], BF16, kind="ExternalInput").ap()
    wv = nc.dram_tensor("wv", [C, P, HPC * HD], BF16, kind="ExternalInput").ap()
    wo = nc.dram_tensor("wo", [OC, P, D], BF16, kind="ExternalInput").ap()
    w1 = nc.dram_tensor("w1", [C, P, HIDC * P], BF16, kind="ExternalInput").ap()
    w2 = nc.dram_tensor("w2", [HIDC, P, D], BF16, kind="ExternalInput").ap()
    mk = None
    if n_mask:
        mk = nc.dram_tensor("mk", [n_mask, P, QW], BF16, kind="ExternalInput").ap()

    ar_in = [nc.dram_tensor(f"ar_in{k}", [D, QW], BF16) for k in range(NCH)]
    ar_out = [
        nc.dram_tensor(f"ar_out{k}", [D, QW], BF16, addr_space="Shared")
        for k in range(NCH)
    ]
    rs_in = [nc.dram_tensor(f"rs_in{k}", [D, QW], BF16) for k in range(NCH)]
    rs_out = [nc.dram_tensor(f"rs_out{k}", [DS, QW], BF16) for k in range(NCH)]
    outs = [
        nc.dram_tensor(f"out{k}", [DS, QW], BF16, kind="ExternalOutput")
        for k in range(NCH)
    ]

    groups = [list(range(N_CORES))]

    with tile.TileContext(nc) as tc, ExitStack() as ctx:
        const = ctx.enter_context(tc.tile_pool(name="const", bufs=1))
        wpool = ctx.enter_context(tc.tile_pool(name="w", bufs=1))
        w1p = ctx.enter_context(tc.tile_pool(name="w1p", bufs=3))
        w2p = ctx.enter_context(tc.tile_pool(name="w2p", bufs=2))
        kvp = ctx.enter_context(tc.tile_pool(name="kv", bufs=1))
        xbp = ctx.enter_context(tc.tile_pool(name="xb", bufs=2))
        htp = ctx.enter_context(tc.tile_pool(name="ht", bufs=2))
        hnp = ctx.enter_context(tc.tile_pool(name="hn", bufs=1))
        upp = ctx.enter_context(tc.tile_pool(name="up", bufs=1))
        qap = ctx.enter_context(tc.tile_pool(name="qa", bufs=2))
        expp = ctx.enter_context(tc.tile_pool(name="exp", bufs=3))
        accp = ctx.enter_context(tc.tile_pool(name="acc", bufs=2))
        sqp = ctx.enter_context(tc.tile_pool(name="sq", bufs=3))
        evp = ctx.enter_context(tc.tile_pool(name="ev", bufs=2))
        stp = ctx.enter_context(tc.tile_pool(name="st", bufs=2))
        psum = ctx.enter_context(tc.tile_pool(name="psum", bufs=1, space="PSUM"))

        ones_f32 = const.tile([P, P], F32)
        nc.vector.memset(ones_f32[:], 1.0)
        ones = const.tile([P, P], BF16)
        nc.vector.tensor_copy(ones[:], ones_f32[:])
        eps_p1 = const.tile([P, 1], F32)
        nc.vector.memset(eps_p1[:], EPS)
        ident = const.tile([P, P], F32)
        make_identity(nc, ident[:])

        # pipeline state carried across iterations
        state = {}

        def load_xb(c):
            b, j = divmod(c, S_TILES)
            xb = xbp.tile([P, C, QW], BF16, tag="xb", name="xb")
            nc.sync.dma_start(
                xb[:],
                xt[b].rearrange("(c p) t -> p c t", p=P)[:, :, ts(j, QW)],
            )
            state[("xb", c)] = xb

        # resident attention weights + mask tiles (xb(0) issued first
        # by the pre-loop below, before these, so PE can start early)
        def load_weights():
            wq_sb = wpool.tile([P, C, HPC * HD], BF16, tag="wq", name="wq_sb")
            nc.sync.dma_start(wq_sb[:], wq.rearrange("c p o -> p c o"))
            wk_sb = wpool.tile([P, C, HPC * HD], BF16, tag="wk", name="wk_sb")
            nc.sync.dma_start(wk_sb[:], wk.rearrange("c p o -> p c o"))
            mtiles = None
            if n_mask:
                mtiles = wpool.tile([P, n_mask, QW], BF16, tag="mk", name="mt")
                nc.sync.dma_start(mtiles[:], mk.rearrange("n p q -> p n q"))
            wv_sb = wpool.tile([P, C, HPC * HD], BF16, tag="wv", name="wv_sb")
            nc.sync.dma_start(wv_sb[:], wv.rearrange("c p o -> p c o"))
            wo_sb = wpool.tile([P, OC, D], BF16, tag="wo", name="wo_sb")
            nc.sync.dma_start(wo_sb[:], wo.rearrange("c p o -> p c o"))
            return wq_sb, wk_sb, wv_sb, wo_sb, mtiles

        def sq_cs(src, cc, cs):
            """One paired square + stats-matmul step (cc-th of C)."""
            sq = sqp.tile([P, QW], BF16, tag="sq", name="sq")
            nc.scalar.activation(sq[:], src[:, cc, :], AF.Square)
            nc.tensor.matmul(
                cs[:], ones[:], sq[:], start=(cc == 0), stop=(cc == C - 1)
            )

        def stats_tail(cs, want_rcol):
            rms = stp.tile([P, QW], F32, tag="rms", name="rms")
            nc.scalar.activation(
                rms[:], cs[:], AF.Sqrt, bias=eps_p1[:], scale=1.0 / D
            )
            rinv = stp.tile([P, QW], F32, tag="rinv", name="rinv")
            nc.vector.reciprocal(rinv[:], rms[:])
            rcol = None
            if want_rcol:
                rcol = stp.tile([P, QW // P], F32, tag="rcol", name="rcol")
                for sub in range(QW // P):
                    tp = psum.tile(
                        [P, QW], F32, tag="score", bufs=2, name="tp"
                    )
                    nc.tensor.transpose(tp[:, :P], rinv[:, ts(sub, P)], ident[:])
                    nc.vector.tensor_copy(rcol[:, sub : sub + 1], tp[:, 0:1])
            return rinv, rcol

        def att_stats_full(c):
            cs = psum.tile([P, QW], F32, tag="cs", bufs=2, name="cs")
            xb = state[("xb", c)]
            for cc in range(C):
                sq_cs(xb, cc, cs)
            rinv, rcol = stats_tail(cs, True)
            state[("rinv", c)] = rinv
            state[("rcol", c)] = rcol

        def att_stats_tail(c):
            cs = state.pop(("cs_att", c))
            rinv, rcol = stats_tail(cs, True)
            state[("rinv", c)] = rinv
            state[("rcol", c)] = rcol

        def attention(c, wq_sb, wk_sb, wv_sb, wo_sb, mtiles):
            b, j = divmod(c, S_TILES)
            if j == 0:
                state["kT"] = kvp.tile([P, HPC, S], BF16, tag="kT", name="kT")
                state["vN"] = kvp.tile(
                    [P, KT, HPC * HD], BF16, tag="vN", name="vN"
                )
            kT, vN = state["kT"], state["vN"]
            xb = state.pop(("xb", c))
            rinv = state.pop(("rinv", c))
            rcol = state.pop(("rcol", c))

            # q/k projections; rinv folded in at eviction
            qTs = qap.tile([P, HPC, QW], BF16, tag="qT", bufs=1, name="qTs")
            for h in range(HPC):
                for w_sb, is_q in ((wq_sb, True), (wk_sb, False)):
                    pp = psum.tile([P, QW], F32, tag="mm", bufs=2, name="pp")
                    for cc in range(C):
                        nc.tensor.matmul(
                            pp[:],
                            w_sb[:, cc, ts(h, HD)],
                            xb[:, cc, :],
                            start=(cc == 0),
                            stop=(cc == C - 1),
                        )
                    if is_q:
                        nc.vector.tensor_mul(qTs[:, h, :], pp[:], rinv[:])
                    else:
                        nc.vector.tensor_mul(kT[:, h, ts(j, QW)], pp[:], rinv[:])
            # v in natural layout; rinv via per-partition scalar
            for sub in range(QW // P):
                pv = psum.tile([P, QW], F32, tag="mm", bufs=2, name="pv")
                for cc in range(C):
                    nc.tensor.matmul(
                        pv[:, : HPC * HD],
                        xb[:, cc, ts(sub, P)],
                        wv_sb[:, cc, :],
                        start=(cc == 0),
                        stop=(cc == C - 1),
                    )
                nc.vector.tensor_scalar_mul(
                    vN[:, j * (QW // P) + sub, :],
                    pv[:, : HPC * HD],
                    rcol[:, sub : sub + 1],
                )

            # flash-style attention for this q-tile; softmax denominator
            # accumulates on the DVE, one ones-matmul per head
            attnT = qap.tile([P, HPC, QW], BF16, tag="attnT", bufs=2, name="at")
            for h in range(HPC):
                kts = [kt for kt in range(KT) if mask_table[(kt, j)] != "skip"]
                pa = psum.tile([P, QW], F32, tag="pv", bufs=2, name="pa")
                acc = accp.tile([P, QW], F32, tag="acc", name="acc")
                n_k = len(kts)
                exs = [None] * n_k

                def _consume(i):
                    kt = kts[i]
                    nc.tensor.matmul(
                        pa[:],
                        vN[:, kt, ts(h, HD)],
                        exs[i][:],
                        start=(i == 0),
                        stop=(i == n_k - 1),
                    )
                    if i == 0:
                        nc.vector.tensor_copy(acc[:], exs[i][:])
                    else:
                        nc.vector.tensor_add(acc[:], acc[:], exs[i][:])

                for i, kt in enumerate(kts):
                    msc = psum.tile(
                        [P, QW], F32, tag="score", bufs=2, name="msc"
                    )
                    nc.tensor.matmul(
                        msc[:],
                        kT[:, h, ts(kt, P)],
                        qTs[:, h, :],
                        start=True,
                        stop=True,
                    )
                    ex = expp.tile([P, QW], BF16, tag="exp", bufs=3, name="ex")
                    nc.scalar.activation(ex[:], msc[:], AF.Exp)
                    mt = mask_table[(kt, j)]
                    if mt != "plain":
                        nc.vector.tensor_mul(ex[:], ex[:], mtiles[:, mt, :])
                    exs[i] = ex
                    if i > 1:
                        _consume(i - 2)
                if n_k > 1:
                    _consume(n_k - 2)
                _consume(n_k - 1)
                accb = accp.tile([P, QW], BF16, tag="accb", name="accb")
                nc.vector.tensor_copy(accb[:], acc[:])
                den = psum.tile([P, QW], F32, tag="cs", bufs=2, name="den")
                nc.tensor.matmul(den[:], ones[:], accb[:], start=True, stop=True)
                rec = stp.tile([P, QW], F32, tag="rec", name="rec")
                nc.vector.reciprocal(rec[:], den[:])
                nc.vector.tensor_mul(attnT[:, h, :], pa[:], rec[:])

            # wo partial + x/8 -> AR input; paired with next chunk's
            # rmsnorm stats to keep the PE dense
            xb_next = state.get(("xb", c + 1))
            cs = None
            if xb_next is not None:
                cs = psum.tile([P, QW], F32, tag="cs", bufs=2, name="cs")
                state[("cs_att", c + 1)] = cs
            for ot in range(C):
                po = psum.tile([P, QW], F32, tag="mm", bufs=2, name="po")
                for oc in range(OC):
                    nc.tensor.matmul(
                        po[:],
                        wo_sb[:, oc, ts(ot, P)],
                        attnT[:, oc, :],
                        start=(oc == 0),
                        stop=(oc == OC - 1),
                    )
                ev = evp.tile([P, QW], BF16, tag="eva", name="eva")
                nc.vector.scalar_tensor_tensor(
                    ev[:], xb[:, ot, :], 1.0 / N_CORES, po[:],
                    op0=mybir.AluOpType.mult, op1=mybir.AluOpType.add,
                )
                nc.gpsimd.dma_start(ar_in[c].ap()[ts(ot, P), :], ev[:])
                if cs is not None:
                    sq_cs(xb_next, ot, cs)

        def load_ht(c):
            ht = htp.tile([P, C, QW], BF16, tag="ht", name="ht")
            nc.sync.dma_start(
                ht[:], ar_out[c].ap().rearrange("(c p) t -> p c t", p=P)
            )
            state[("ht", c)] = ht

        def ffn_stats_tail(c):
            if ("cs_ffn", c) not in state:
                # chunk 0: no preceding ffn_body to fuse the stats into
                cs0 = psum.tile([P, QW], F32, tag="cs", bufs=2, name="cs0")
                ht0 = state[("ht", c)]
                for cc in range(C):
                    sq_cs(ht0, cc, cs0)
                state[("cs_ffn", c)] = cs0
            cs = state.pop(("cs_ffn", c))
            r2, _ = stats_tail(cs, False)
            ht = state[("ht", c)]
            hn = hnp.tile([P, C, QW], BF16, tag="hn", name="hn")
            for cc in range(C):
                nc.vector.tensor_mul(hn[:, cc, :], ht[:, cc, :], r2[:])
            state[("hn", c)] = hn

        def load_w1(i):
            w1t = w1p.tile([P, C, P], BF16, tag="w1s", name="w1t")
            nc.gpsimd.dma_start(
                w1t[:], w1.rearrange("c p o -> p c o")[:, :, ts(i, P)]
            )
            return w1t

        def load_w2(q):
            w2t = w2p.tile([P, HIDC, QW], BF16, tag="w2q", name="w2t")
            nc.gpsimd.dma_start(
                w2t[:], w2.rearrange("c p o -> p c o")[:, :, ts(q, QW)]
            )
            return w2t

        def ffn_body(c):
            ht = state.pop(("ht", c))
            hn = state.pop(("hn", c))
            w1ts = [load_w1(0), load_w1(1)]
            w2ts = [load_w2(0)]
            up = upp.tile([P, HIDC, QW], BF16, tag="up", name="up")
            for ht_i in range(HIDC):
                if ht_i + 2 < HIDC:
                    w1ts.append(load_w1(ht_i + 2))
                pu = psum.tile([P, QW], F32, tag="mm", bufs=2, name="pu")
                for cc in range(C):
                    nc.tensor.matmul(
                        pu[:],
                        w1ts[ht_i][:, cc, :],
                        hn[:, cc, :],
                        start=(cc == 0),
                        stop=(cc == C - 1),
                    )
                nc.scalar.activation(up[:, ht_i, :], pu[:], AF.Relu)
            # w2 partial + h/8 -> RS input; paired with next chunk's
            # ffn rmsnorm stats
            ht_next = state.get(("ht", c + 1))
            cs = None
            if ht_next is not None:
                cs = psum.tile([P, QW], F32, tag="cs", bufs=2, name="cs")
                state[("cs_ffn", c + 1)] = cs
            for q in range(W2Q):
                if q + 1 < W2Q:
                    w2ts.append(load_w2(q + 1))
                for o4 in range(C // W2Q):
                    ot = q * (C // W2Q) + o4
                    pd = psum.tile([P, QW], F32, tag="mm", bufs=2, name="pd")
                    for hc in range(HIDC):
                        nc.tensor.matmul(
                            pd[:],
                            w2ts[q][:, hc, ts(o4, P)],
                            up[:, hc, :],
                            start=(hc == 0),
                            stop=(hc == HIDC - 1),
                        )
                    ev = evp.tile([P, QW], BF16, tag="evf", name="evf")
                    nc.vector.scalar_tensor_tensor(
                        ev[:], ht[:, ot, :], 1.0 / N_CORES, pd[:],
                        op0=mybir.AluOpType.mult, op1=mybir.AluOpType.add,
                    )
                    nc.gpsimd.dma_start(rs_in[c].ap()[ts(ot, P), :], ev[:])
                    if cs is not None:
                        sq_cs(ht_next, ot, cs)

        # ---- software-pipelined main loop ----
        load_xb(0)
        wq_sb, wk_sb, wv_sb, wo_sb, mtiles = load_weights()
        att_stats_full(0)
        for c in range(NCH + LAG):
            if c + 1 < NCH:
                load_xb(c + 1)
            if 0 <= c - 1 < NCH:
                load_ht(c - 1)
            if c < NCH:
                attention(c, wq_sb, wk_sb, wv_sb, wo_sb, mtiles)
                nc.gpsimd.collective_compute(
                    "AllReduce",
                    mybir.AluOpType.add,
                    replica_groups=groups,
                    ins=[ar_in[c].ap().opt()],
                    outs=[ar_out[c].ap().opt()],
                )
            if c - LAG >= 0:
                f = c - LAG
                ffn_body(f)
                nc.gpsimd.collective_compute(
                    "ReduceScatter",
                    mybir.AluOpType.add,
                    replica_groups=groups,
                    ins=[rs_in[f].ap().opt()],
                    outs=[rs_out[f].ap().opt()],
                )
                nc.sync.dma_start(outs[f].ap(), rs_out[f].ap())
            if c + 1 < NCH:
                att_stats_tail(c + 1)
            if 0 <= c - 1 < NCH:
                ffn_stats_tail(c - 1)

    nc.compile()
    return nc, NCH, DS


_CACHE = {}
LAST_RESULT = None


def _get_program(B, S, D, H, HID, mask_table, n_mask, mask_key):
    key = (B, S, D, H, HID, mask_key)
    if key not in _CACHE:
        _CACHE[key] = build_program(B, S, D, H, HID, mask_table, n_mask)
    return _CACHE[key]


def kernel(x, mask, wq, wk, wv, wo, w1, w2, attn_norm_w, ffn_norm_w):
    x = np.asarray(x, dtype=np.float32)
    mask = np.asarray(mask, dtype=np.float32)
    wq, wk, wv, wo = (np.asarray(a, dtype=np.float32) for a in (wq, wk, wv, wo))
    w1, w2 = np.asarray(w1, dtype=np.float32), np.asarray(w2, dtype=np.float32)
    attn_norm_w = np.asarray(attn_norm_w, dtype=np.float32)
    ffn_norm_w = np.asarray(ffn_norm_w, dtype=np.float32)

    B, S, D = x.shape
    H = D // 128  # HD is fixed at 128 (= SBUF partition count)
    HID = w1.shape[0]
    HD = D // H
    HPC = H // N_CORES
    C = D // P
    HIDC = HID // N_CORES // P

    mask_table, mtiles_np = _classify_mask(
        np.broadcast_to(mask, (1, 1, S, S))[0, 0], S
    )
    mask_key = hash(tuple(sorted((k, str(v)) for k, v in mask_table.items())))
    nc, NCH, DS = _get_program(
        B, S, D, H, HID, mask_table, len(mtiles_np), mask_key
    )

    # ---- host-side prep ----
    xt = np.ascontiguousarray(x.transpose(0, 2, 1)).astype(BF16_NP)  # [B, D, S]
    wq_f = (wq * attn_norm_w[None, :]) / np.sqrt(HD)
    wk_f = wk * attn_norm_w[None, :]
    w1_f = w1 * ffn_norm_w[None, :]

    in_maps = []
    for c in range(N_CORES):
        hs = slice(c * HPC * HD, (c + 1) * HPC * HD)
        qs = np.ascontiguousarray(wq_f[hs].T).reshape(C, P, HPC * HD).astype(BF16_NP)
        ks = np.ascontiguousarray(wk_f[hs].T).reshape(C, P, HPC * HD).astype(BF16_NP)
        vs = np.ascontiguousarray(wv[hs].T).reshape(C, P, HPC * HD).astype(BF16_NP)
        os_ = np.ascontiguousarray(wo[:, hs].T).reshape(HPC, P, D).astype(BF16_NP)
        fs = slice(c * HIDC * P, (c + 1) * HIDC * P)
        w1s = np.ascontiguousarray(w1_f[fs].T).reshape(C, P, HIDC * P).astype(BF16_NP)
        w2r = (
            np.ascontiguousarray(w2[:, fs].T)
            .reshape(HIDC, P, D)
            .astype(BF16_NP)
        )
        m = {
            "xt": xt,
            "wq": qs,
            "wk": ks,
            "wv": vs,
            "wo": os_,
            "w1": w1s,
            "w2": w2r,
        }
        if len(mtiles_np):
            m["mk"] = np.stack(mtiles_np).astype(BF16_NP)
        in_maps.append(m)

    trace = os.environ.get("KTRACE", "0") == "1"
    res = run_bass_kernel_spmd(nc, in_maps, list(range(N_CORES)), trace=trace)
    global LAST_RESULT
    LAST_RESULT = res

    out_T = np.empty((D, B * S), dtype=np.float32)
    for r_ in range(N_CORES):
        for k in range(NCH):
            out_T[r_ * DS : (r_ + 1) * DS, k * QW : (k + 1) * QW] = np.asarray(
                res.results[r_][f"out{k}"], dtype=np.float32
            )
    return np.ascontiguousarray(out_T.reshape(D, B, S).transpose(1, 2, 0))
